# revision 42
# baseline (speedup 1.0000x reference)
"""Trainium2 Bass kernel for AuxiliaryMultiHeadedAttention (v2).

Reference computation (B=4, L=2048, H=256, NH=8, DH=32):
    kb   = split_heads(k_b @ Wb.T + bb)
    corr = (qh @ kh^T + qh @ kb^T) / sqrt(DH) * scale_w[h, q]
    corr = where(mask==0, -1e9, corr);  prob = softmax(corr)
    out  = merge_heads(prob @ vh) @ Ww.T + bw

Kernel strategy (8 NeuronCores):
    Shard (batch, query-half): core c -> batch c//2, queries (c%2)*1024..+1024.

    v2 changes vs v1 (253.8us -> ~215us):
      * Main-loop S matmuls in bf16 (keffT/qsT/WbT/kbT bf16): same
        1 cyc/row as f32r, faster LDWEIGHTS, lower PE power.
      * exp split across engines by kc parity: even kc tiles on ACT (true
        exp), odd kc tiles on DVE via a Schraudolph bit-trick
        (int16(184.66*x + 16249) bitcast to bf16 ~= e^x, sigma=-7 tuned so
        the estimator is unbiased vs ACT's exp — the bias must cancel
        between the interleaved key chunks or softmax mass shifts).
      * Depth-5 software pipeline [S(kc); exp(kc); PV(kc-5)] (st tiles are
        freed by exp, not PV, so PV depth is free in PSUM) reduces
        exp->PV stalls that re-trip the HAM clock gate to K=4/8 (1.2 GHz).
      * PSUM: st bufs=3 (2 banks each) + pv bufs=2 (1 bank) = 8 banks.
      * normalize: denominator copied PSUM->SBUF on ACT first
        (reciprocal_approx_fast misreads PSUM operands on HW, sim-only
        correct!), reciprocal+mul on DVE in the next group's idle slots.
      * PE kept busy through prep DMA waits and the prep->main PSUM bank
        handoff with dummy ident transposes (HAM warmup); input DMAs
        split across SP and ACT queues; per-bank keff evacuation.
"""

import sys

if "/opt/trn_rl_repo" not in sys.path:
    sys.path.insert(0, "/opt/trn_rl_repo")

import math

import numpy as np

B, L, H, NH, DH = 4, 2048, 256, 8, 32
LQ = 1024  # queries per core
NCORES = 8
ISQ = 1.0 / math.sqrt(DH)
# Schraudolph constants for bf16 bit-trick exp on DVE:
#   P = bitcast_bf16(int16(A*x + 127*128 + sigma)) ~= e^x
SCH_A = 128.0 / math.log(2.0)
SCH_SIGMA = -7.0
SCH_B = 127.0 * 128.0 + SCH_SIGMA
N_WARMUP = 56  # dummy PE transposes to hold the HAM clock gate open


def _build():
    import concourse.bass as bass  # noqa: F401
    import concourse.mybir as mybir
    import concourse.tile as tile
    from concourse import bacc

    f32 = mybir.dt.float32
    f32r = mybir.dt.float32r
    i32 = mybir.dt.int32
    i16 = mybir.dt.int16
    bf16 = mybir.dt.bfloat16
    Exp = mybir.ActivationFunctionType.Exp
    Cp = mybir.ActivationFunctionType.Copy
    Alu = mybir.AluOpType

    nc = bacc.Bacc("TRN2", target_bir_lowering=False, debug=False, num_devices=NCORES)

    q_d = nc.dram_tensor("q_s", [LQ, H], f32, kind="ExternalInput")
    k_d = nc.dram_tensor("k_s", [L, H], f32, kind="ExternalInput")
    v_d = nc.dram_tensor("v_s", [L, H], f32, kind="ExternalInput")
    kb_d = nc.dram_tensor("kb_s", [L, H], f32, kind="ExternalInput")
    mask_d = nc.dram_tensor("mask_s", [L], i32, kind="ExternalInput")
    sw_d = nc.dram_tensor("sw_s", [NH, LQ], f32, kind="ExternalInput")
    Wb_d = nc.dram_tensor("Wb", [H, H], f32, kind="ExternalInput")
    bb_d = nc.dram_tensor("bb", [H], f32, kind="ExternalInput")
    Ww_d = nc.dram_tensor("Ww", [H, H], f32, kind="ExternalInput")
    bw_d = nc.dram_tensor("bw", [H], f32, kind="ExternalInput")
    id_d = nc.dram_tensor("ident", [128, 128], f32, kind="ExternalInput")
    out_d = nc.dram_tensor("out", [LQ, H], f32, kind="ExternalOutput")
    DBG = False
    if DBG:
        dbg_keff = [nc.dram_tensor(f"dbg_keff{g}", [128, L], f32,
                                   kind="ExternalOutput") for g in range(2)]
        dbg_qs = [nc.dram_tensor(f"dbg_qs{g}", [128, LQ], f32,
                                 kind="ExternalOutput") for g in range(2)]
        dbg_vmm = nc.dram_tensor("dbg_vmm", [128, 16 * NH * 64], bf16,
                                 kind="ExternalOutput")
        dbg_hid = [nc.dram_tensor(f"dbg_hid{g}", [128, LQ], f32,
                                  kind="ExternalOutput") for g in range(2)]

    copy_flip = [0]

    with tile.TileContext(nc) as tc:
        with (
            tc.tile_pool(name="persist", bufs=1) as pp,
            tc.tile_pool(name="pt", bufs=7) as ptp,
            tc.tile_pool(name="small", bufs=4) as smp,
        ):
            # ---------------- persistent SBUF tensors ----------------
            ident = pp.tile([128, 128], f32, tag="ident")
            nc.sync.dma_start(out=ident, in_=id_d[:, :])
            keffT = [pp.tile([128, L], bf16, tag=f"keffT{g}", name=f"keffT{g}")
                     for g in range(2)]
            qsT = [pp.tile([128, LQ], bf16, tag=f"qsT{g}", name=f"qsT{g}")
                   for g in range(2)]
            # per (key-chunk, head): [v_hi | mask] -> [128, 64] bf16
            vmm = pp.tile([128, 16 * NH * 64], bf16, tag="vmm")
            hidT = [pp.tile([128, LQ], bf16, tag=f"hidT{g}", name=f"hidT{g}")
                    for g in range(2)]
            WwT = [pp.tile([128, H], bf16, tag=f"WwT{g}", name=f"WwT{g}")
                   for g in range(2)]
            ones1 = pp.tile([1, 128], f32, tag="ones1")
            nc.vector.memset(ones1, 1.0)
            ones1r = pp.tile([1, 128], f32r, tag="ones1r")
            nc.vector.tensor_copy(ones1r, ones1)
            bwr = pp.tile([1, H], f32r, tag="bwr")
            sc8 = pp.tile([128, 64], f32, tag="sc8")
            outsb = pp.tile([128, 8 * H], f32, tag="outsb")
            bbr = pp.tile([1, H], bf16, tag="bbr")
            oneslr = pp.tile([1, L], bf16, tag="oneslr")
            nc.vector.memset(oneslr, 1.0)

            with tc.tile_pool(name="stage", bufs=1) as sp:
                def pcopy(dst, src):
                    # alternate psum->sbuf evacuation between DVE and ACT
                    if copy_flip[0] % 2 == 0:
                        nc.vector.tensor_copy(dst, src)
                    else:
                        nc.scalar.copy(dst, src)
                    copy_flip[0] += 1

                # warm the ACT exp table before the main loop needs it
                dummy = sp.tile([1, 128], f32, tag="dummy")
                nc.vector.memset(dummy, 0.0)
                dummy2 = sp.tile([1, 128], f32, tag="dummy2")
                nc.scalar.activation(dummy2, dummy, Exp)

                # ---------------- staging loads (critical path first) ----
                m16 = sp.tile([16, 128], i32, tag="m16")
                nc.sync.dma_start(out=m16,
                                  in_=mask_d.rearrange("(c p) -> c p", p=128))
                swt = sp.tile([NH, LQ], f32, tag="swt")
                nc.sync.dma_start(out=swt, in_=sw_d[:, :])
                wbraw = sp.tile([128, 2 * H], f32, tag="wbraw")
                nc.sync.dma_start(out=wbraw.rearrange("p (c e) -> p c e", c=2),
                                  in_=Wb_d.rearrange("(c p) e -> p c e", p=128))
                qraw = sp.tile([128, 8 * H], f32, tag="qraw")
                nc.sync.dma_start(out=qraw.rearrange("p (c e) -> p c e", c=8),
                                  in_=q_d.rearrange("(c p) e -> p c e", p=128))
                bbt = sp.tile([1, H], f32, tag="bbt")
                nc.sync.dma_start(out=bbt, in_=bb_d[None, :])
                nc.vector.tensor_copy(bbr, bbt)
                kbraw = sp.tile([128, 16 * H], f32, tag="kbraw")
                kraw = sp.tile([128, 16 * H], f32, tag="kraw")
                vraw = sp.tile([128, 16 * H], f32, tag="vraw")

                def load4(tile_, dram, c4, eng=None):
                    tv = tile_.rearrange("p (c e) -> p c e", c=16)
                    dv = dram.rearrange("(c p) e -> p c e", p=128)
                    (eng or nc.sync).dma_start(
                        out=tv[:, c4 * 4:(c4 + 1) * 4, :],
                        in_=dv[:, c4 * 4:(c4 + 1) * 4, :])

                for c4 in range(4):
                    load4(kraw, k_d, c4, eng=nc.scalar)
                for c4 in range(4):
                    load4(kbraw, kb_d, c4)
                for c4 in range(4):
                    load4(vraw, v_d, c4, eng=nc.scalar)
                wwraw = sp.tile([128, 2 * H], f32, tag="wwraw")
                nc.scalar.dma_start(out=wwraw.rearrange("p (c e) -> p c e", c=2),
                                    in_=Ww_d.rearrange("(c p) e -> p c e", p=128))
                bwt = sp.tile([1, H], f32, tag="bwt")
                nc.scalar.dma_start(out=bwt, in_=bw_d[None, :])
                nc.vector.tensor_copy(bwr, bwt)
                m16f = sp.tile([16, 128], f32, tag="m16f")
                nc.vector.tensor_copy(m16f, m16)
                maskf = sp.tile([128, 16], f32, tag="maskf")
                WbT = [sp.tile([128, H], bf16, tag=f"WbT{e}", name=f"WbT{e}")
                       for e in range(2)]
                kbT = [sp.tile([128, L], bf16, tag=f"kbT{e}", name=f"kbT{e}")
                       for e in range(2)]

                # ---------------- prep: transposes & keff ----------------
                with (
                    tc.tile_pool(name="ptr", bufs=4, space="PSUM") as ptr,
                    tc.tile_pool(name="pkeff", bufs=1, space="PSUM") as pkf,
                ):
                    # PE warmup: keep the HAM activity monitor busy during
                    # the DMA head so the clock gate opens (and stays open)
                    # before the real transpose burst.
                    for w in range(N_WARMUP):
                        t = ptr.tile([128, 128], f32, tag="tr", name="warm")
                        nc.tensor.transpose(t, ident, ident)

                    # mask -> maskf [128, 16]
                    tm = ptr.tile([128, 16], f32, tag="tr", name="tm")
                    nc.tensor.transpose(tm, m16f, ident[0:16, 0:16])
                    nc.vector.tensor_copy(maskf, tm)

                    # scale_w slices -> sc8 [128, 8 per q-chunk]
                    for mq in range(8):
                        t = ptr.tile([128, 8], f32, tag="tr", name="t")
                        nc.tensor.transpose(t, swt[:, mq * 128:(mq + 1) * 128],
                                            ident[0:NH, 0:NH])
                        nc.vector.tensor_copy(sc8[:, mq * 8:(mq + 1) * 8], t)

                    # Wb transposes -> WbT bf16
                    for dc in range(2):
                        for ec in range(2):
                            t = ptr.tile([128, 128], f32, tag="tr", name="t")
                            nc.tensor.transpose(
                                t,
                                wbraw[:, dc * H + ec * 128: dc * H + (ec + 1) * 128],
                                ident)
                            pcopy(WbT[ec][:, dc * 128:(dc + 1) * 128], t)

                    for _w in range(14):
                        t = ptr.tile([128, 128], f32, tag="tr", name="warm")
                        nc.tensor.transpose(t, ident, ident)
                    # k_b transpose -> kbT bf16 (4 transposes per
                    # psum tile, one wide evacuation copy each)
                    for lb in range(4):
                        for ec in range(2):
                            tb = ptr.tile([128, 512], f32, tag="tr",
                                          name="tb")
                            for j in range(4):
                                lc = lb * 4 + j
                                nc.tensor.transpose(
                                    tb[:, j * 128:(j + 1) * 128],
                                    kbraw[:, lc * H + ec * 128:
                                          lc * H + (ec + 1) * 128],
                                    ident)
                            pcopy(kbT[ec][:, lb * 512:(lb + 1) * 512], tb)

                    # q: scale by scale_w/sqrt(DH) (DVE, in place)
                    for mq in range(8):
                        qv = qraw[:, mq * H:(mq + 1) * H].rearrange(
                            "p (h j) -> p h j", h=NH)
                        nc.vector.scalar_tensor_tensor(
                            out=qv, in0=qv, scalar=ISQ,
                            in1=sc8[:, mq * 8:(mq + 1) * 8][:, :, None].broadcast_to(
                                [128, 8, 32]),
                            op0=Alu.mult, op1=Alu.mult)

                    def keff_mms_first(dc, pk):
                        for ns in range(4):
                            for ec in range(2):
                                nc.tensor.matmul(
                                    pk[:, ns * 512:(ns + 1) * 512],
                                    lhsT=WbT[ec][:, dc * 128:(dc + 1) * 128],
                                    rhs=kbT[ec][:, ns * 512:(ns + 1) * 512],
                                    start=(ec == 0), stop=False)
                            nc.tensor.matmul(
                                pk[:, ns * 512:(ns + 1) * 512],
                                lhsT=bbr[0:1, dc * 128:(dc + 1) * 128],
                                rhs=oneslr[0:1, ns * 512:(ns + 1) * 512],
                                start=False, stop=False)

                    def keff_transposes(dc, pk):
                        # accumulate k^T on top of kb@Wb^T + bb; per-bank
                        # stop + immediate evacuation
                        for ns in range(4):
                            for j in range(4):
                                lc = ns * 4 + j
                                nc.tensor.matmul(
                                    pk[:, lc * 128:(lc + 1) * 128],
                                    lhsT=kraw[:, lc * H + dc * 128:
                                              lc * H + (dc + 1) * 128],
                                    rhs=ident,
                                    is_transpose=True,
                                    start=False, stop=(j == 3))
                            pcopy(keffT[dc][:, ns * 512:(ns + 1) * 512],
                                  pk[:, ns * 512:(ns + 1) * 512])

                    def warm(n):
                        for _ in range(n):
                            t = ptr.tile([128, 128], f32, tag="tr",
                                         name="warm")
                            nc.tensor.transpose(t, ident, ident)

                    def qtrans(mqs):
                        mqs = list(mqs)
                        for dc in range(2):
                            for b in range(0, len(mqs), 4):
                                blk = mqs[b:b + 4]
                                tb = ptr.tile([128, 512], f32, tag="tr",
                                              name="tb")
                                for j, mq in enumerate(blk):
                                    nc.tensor.transpose(
                                        tb[:, j * 128:(j + 1) * 128],
                                        qraw[:, mq * H + dc * 128:
                                             mq * H + (dc + 1) * 128],
                                        ident)
                                pcopy(qsT[dc][:, blk[0] * 128:
                                              (blk[-1] + 1) * 128], tb)

                    warm(12)
                    pk0 = pkf.tile([128, L], f32, tag="pk", name="pk0")
                    keff_mms_first(0, pk0)
                    warm(8)
                    keff_transposes(0, pk0)

                    qtrans(range(4))
                    pk1 = pkf.tile([128, L], f32, tag="pk", name="pk1")
                    keff_mms_first(1, pk1)
                    warm(6)
                    keff_transposes(1, pk1)

                    # tail PE work (not needed until later in the main
                    # loop) keeps HAM warm through the psum-bank handoff
                    qtrans(range(4, 8))
                    for er in range(2):
                        for g in range(2):
                            t = ptr.tile([128, 128], f32, tag="tr", name="t")
                            nc.tensor.transpose(
                                t,
                                wwraw[:, er * H + g * 128: er * H + (g + 1) * 128],
                                ident)
                            pcopy(WwT[g][:, er * 128:(er + 1) * 128], t)
                    warm(8)

                    # vmm build late so ACT's critical pcopies are not
                    # queued behind it; mask-reps on the idle Pool engine
                    vmm4 = vmm.rearrange("p (c h w) -> p c h w", c=16, h=NH)
                    vraw3 = vraw.rearrange("p (c e) -> p c e", c=16)
                    for lc in range(16):
                        vsl = vraw3[:, lc, :].rearrange("p (h j) -> p h j", h=NH)
                        nc.scalar.activation(vmm4[:, lc, :, 0:32], vsl, Cp,
                                             scale=maskf[:, lc:lc + 1])
                        nc.gpsimd.tensor_copy(
                            vmm4[:, lc, :, 32:64],
                            maskf[:, lc:lc + 1][:, :, None].broadcast_to(
                                [128, NH, 32]))

            # ---------------- main attention loop ----------------
            # group (g, qb): heads (2g, 2g+1), queries qb*512..+512
            # per kc: [PV(kc-3); S(kc); exp(kc)] depth-3 pipeline.
            # exp engine alternates by kc parity: even=ACT exp, odd=DVE
            # Schraudolph. Normalization of the previous group's pv is
            # emitted into the DVE-idle even-kc slots.
            with (
                tc.tile_pool(name="pst", bufs=3, space="PSUM") as pst,
                tc.tile_pool(name="ppv", bufs=2, space="PSUM") as ppv,
            ):
                groups = [(g, qb) for g in range(4) for qb in range(2)]
                prev_norm = []

                for g, qb in groups:
                    ch = g // 2
                    pv = ppv.tile([128, 512], f32, tag="pv", name=f"pv{g}_{qb}")
                    pts = {}

                    def emit_pv(kc, pv=pv, pts=pts, g=g):
                        for t in range(2):
                            h = 2 * g + t
                            nc.tensor.matmul(
                                pv[64 * t:64 * t + 64, :],
                                lhsT=vmm[:, (kc * NH + h) * 64:
                                         (kc * NH + h) * 64 + 64],
                                rhs=pts[kc][:, t * 512:(t + 1) * 512],
                                tile_position=(0, 64 * t),
                                start=(kc == 0), stop=(kc == 15),
                                skip_group_check=True)

                    for kc in range(16):
                        st = pst.tile([128, 1024], f32, tag="st", name="st")
                        for t in range(2):
                            ro = (g % 2) * 64 + t * 32
                            nc.tensor.matmul(
                                st[:, t * 512:(t + 1) * 512],
                                lhsT=keffT[ch][ro:ro + 32,
                                               kc * 128:(kc + 1) * 128],
                                rhs=qsT[ch][ro:ro + 32,
                                            qb * 512:(qb + 1) * 512],
                                tile_position=(ro, 0),
                                start=True, stop=True)
                        pt = ptp.tile([128, 1024], bf16, tag="pt", name="pt")
                        if kc % 2 == 0:
                            nc.scalar.activation(pt, st, Exp)
                        else:
                            nc.vector.tensor_scalar(
                                out=pt.bitcast(i16), in0=st,
                                scalar1=SCH_A, scalar2=SCH_B,
                                op0=Alu.mult, op1=Alu.add)
                        pts[kc] = pt
                        nc.tensor.ldweights(
                            weights=keffT[ch][0:32, 0:128])
                        if kc >= 5:
                            emit_pv(kc - 5)
                        # previous group's normalize in DVE-idle even slots
                        if prev_norm and kc in (1, 3, 5, 7):
                            prev_norm[kc // 2]()
                    for kc in (11, 12, 13, 14, 15):
                        emit_pv(kc)

                    def make_norm(pv=pv, g=g, qb=qb, ch=ch):
                        # reciprocal_approx_fast misreads PSUM operands on
                        # HW (sim-only correct) — stage the denominator
                        # through SBUF via an ACT copy first.
                        state = {}
                        pieces = []
                        for t in range(2):
                            ro = (g % 2) * 64 + t * 32

                            def piece_a(pv=pv, t=t, state=state):
                                rs = smp.tile([32, 512], f32, tag="rsum",
                                              name="rsum")
                                nc.scalar.copy(
                                    rs, pv[64 * t + 32:64 * t + 64, :])
                                r = smp.tile([32, 512], f32, tag="rcp",
                                             name="rcp")
                                nc.vector.reciprocal_approx_fast(r, rs)
                                state[t] = r

                            def piece_b(pv=pv, t=t, ro=ro, ch=ch, qb=qb,
                                        state=state):
                                nc.vector.tensor_mul(
                                    hidT[ch][ro:ro + 32,
                                             qb * 512:(qb + 1) * 512],
                                    pv[64 * t:64 * t + 32, :], state[t])

                            pieces.append(piece_a)
                            pieces.append(piece_b)
                        return pieces

                    prev_norm = make_norm()

                # final group's normalize
                for p in prev_norm:
                    p()

            if DBG:
                for g in range(2):
                    nc.sync.dma_start(out=dbg_keff[g][:, :],
                                      in_=keffT[g].bitcast(f32))
                    nc.sync.dma_start(out=dbg_qs[g][:, :],
                                      in_=qsT[g].bitcast(f32))
                    nc.sync.dma_start(out=dbg_hid[g][:, :],
                                      in_=hidT[g].bitcast(f32))
                nc.sync.dma_start(out=dbg_vmm[:, :], in_=vmm)

            # ---------------- output linear ----------------
            with tc.tile_pool(name="pout", bufs=2, space="PSUM") as pout:
                for mq in range(8):
                    po = pout.tile([128, H], f32, tag="po", name="po")
                    for g in range(2):
                        nc.tensor.matmul(
                            po,
                            lhsT=hidT[g][:, mq * 128:(mq + 1) * 128],
                            rhs=WwT[g],
                            start=(g == 0), stop=False)
                    nc.tensor.matmul(
                        po, lhsT=ones1r, rhs=bwr, start=False, stop=True)
                    pcopy(outsb[:, mq * H:(mq + 1) * H], po)
                    if mq % 2 == 1:
                        nc.sync.dma_start(
                            out=out_d.rearrange("(c p) e -> p c e", p=128)[
                                :, mq - 1:mq + 1, :],
                            in_=outsb.rearrange("p (c e) -> p c e", c=8)[
                                :, mq - 1:mq + 1, :])

    nc.compile()
    return nc


def _make_in_maps(inputs):
    q = np.ascontiguousarray(np.asarray(inputs["q"], dtype=np.float32))
    k = np.ascontiguousarray(np.asarray(inputs["k"], dtype=np.float32))
    v = np.ascontiguousarray(np.asarray(inputs["v"], dtype=np.float32))
    k_b = np.ascontiguousarray(np.asarray(inputs["k_b"], dtype=np.float32))
    mask = np.ascontiguousarray(np.asarray(inputs["mask"], dtype=np.int32))
    sw = np.ascontiguousarray(np.asarray(inputs["scale_w"], dtype=np.float32))
    Wb = np.ascontiguousarray(np.asarray(inputs["Wb"], dtype=np.float32))
    bb = np.ascontiguousarray(np.asarray(inputs["bb"], dtype=np.float32))
    Ww = np.ascontiguousarray(np.asarray(inputs["Ww"], dtype=np.float32))
    bw = np.ascontiguousarray(np.asarray(inputs["bw"], dtype=np.float32))
    ident = np.eye(128, dtype=np.float32)
    in_maps = []
    for c in range(NCORES):
        b, qs = c // 2, c % 2
        in_maps.append({
            "q_s": q[b, qs * LQ:(qs + 1) * LQ, :],
            "k_s": k[b],
            "v_s": v[b],
            "kb_s": k_b[b],
            "mask_s": mask[b],
            "sw_s": np.ascontiguousarray(sw[:, qs * LQ:(qs + 1) * LQ]),
            "Wb": Wb, "bb": bb, "Ww": Ww, "bw": bw,
            "ident": ident,
        })
    return in_maps


def run_sharded(inputs, trace=False, tmpdir=None):
    from concourse.bass_utils import run_bass_kernel_spmd
    from concourse import bass_utils

    if trace:
        _install_ntff_hook()
        bass_utils.upload_artifacts = lambda d: d
    nc = _build()
    in_maps = _make_in_maps(inputs)
    res = run_bass_kernel_spmd(nc, in_maps, list(range(NCORES)),
                               trace=trace, tmpdir=tmpdir)
    out = np.empty((B, L, H), dtype=np.float32)
    for c in range(NCORES):
        b, qs = c // 2, c % 2
        out[b, qs * LQ:(qs + 1) * LQ, :] = res.results[c]["out"]
    return out, res


def kernel(**inputs):
    out, _ = run_sharded(inputs, trace=False)
    return out


def _install_ntff_hook():
    """Provide antenv.axon_hooks (absent in this image) so trace=True works."""
    import contextlib
    import ctypes
    import types

    import antenv

    if hasattr(antenv, "axon_hooks"):
        return
    mod = types.ModuleType("antenv.axon_hooks")
    _hook = [None]
    mod.set_axon_ntff_profile_hook = lambda h: _hook.__setitem__(0, h)
    mod.get_axon_ntff_profile_hook = lambda: _hook[0]
    antenv.axon_hooks = mod
    sys.modules["antenv.axon_hooks"] = mod

    lib = ctypes.CDLL("/opt/axon/libaxon_pjrt.so")
    if not hasattr(lib, "axon_start_nrt_profile"):
        return
    lib.axon_start_nrt_profile.argtypes = [ctypes.POINTER(ctypes.c_int64),
                                           ctypes.c_size_t]
    lib.axon_start_nrt_profile.restype = ctypes.c_int64
    lib.axon_stop_nrt_profile.argtypes = [ctypes.c_char_p]
    lib.axon_stop_nrt_profile.restype = ctypes.c_int64

    @contextlib.contextmanager
    def _profile(output_dir, device_ids):
        import jax

        jax.devices()
        if device_ids:
            ids = (ctypes.c_int64 * len(device_ids))(*device_ids)
            rc = lib.axon_start_nrt_profile(ids, len(device_ids))
        else:
            rc = lib.axon_start_nrt_profile(None, 0)
        if rc != 0:
            raise RuntimeError(f"axon_start_nrt_profile rc={rc}")
        try:
            yield
        finally:
            n = lib.axon_stop_nrt_profile(str(output_dir).encode())
            print(f"profile: {n} file(s) written to {output_dir}",
                  file=sys.stderr)

    mod.set_axon_ntff_profile_hook(_profile)


# revision 43
# speedup vs baseline: 1.0129x; 1.0129x over previous
"""Trainium2 Bass kernel for AuxiliaryMultiHeadedAttention (v2).

Reference computation (B=4, L=2048, H=256, NH=8, DH=32):
    kb   = split_heads(k_b @ Wb.T + bb)
    corr = (qh @ kh^T + qh @ kb^T) / sqrt(DH) * scale_w[h, q]
    corr = where(mask==0, -1e9, corr);  prob = softmax(corr)
    out  = merge_heads(prob @ vh) @ Ww.T + bw

Kernel strategy (8 NeuronCores):
    Shard (batch, query-half): core c -> batch c//2, queries (c%2)*1024..+1024.

    v2 changes vs v1 (253.8us -> ~215us):
      * Main-loop S matmuls in bf16 (keffT/qsT/WbT/kbT bf16): same
        1 cyc/row as f32r, faster LDWEIGHTS, lower PE power.
      * exp split across engines by kc parity: even kc tiles on ACT (true
        exp), odd kc tiles on DVE via a Schraudolph bit-trick
        (int16(184.66*x + 16249) bitcast to bf16 ~= e^x, sigma=-7 tuned so
        the estimator is unbiased vs ACT's exp — the bias must cancel
        between the interleaved key chunks or softmax mass shifts).
      * Depth-5 software pipeline [S(kc); exp(kc); PV(kc-5)] (st tiles are
        freed by exp, not PV, so PV depth is free in PSUM) reduces
        exp->PV stalls that re-trip the HAM clock gate to K=4/8 (1.2 GHz).
      * PSUM: st bufs=3 (2 banks each) + pv bufs=2 (1 bank) = 8 banks.
      * normalize: denominator copied PSUM->SBUF on ACT first
        (reciprocal_approx_fast misreads PSUM operands on HW, sim-only
        correct!), reciprocal+mul on DVE in the next group's idle slots.
      * PE kept busy through prep DMA waits and the prep->main PSUM bank
        handoff with dummy ident transposes (HAM warmup); input DMAs
        split across SP and ACT queues; per-bank keff evacuation.
"""

import sys

if "/opt/trn_rl_repo" not in sys.path:
    sys.path.insert(0, "/opt/trn_rl_repo")

import math

import numpy as np

B, L, H, NH, DH = 4, 2048, 256, 8, 32
LQ = 1024  # queries per core
NCORES = 8
ISQ = 1.0 / math.sqrt(DH)
# Schraudolph constants for bf16 bit-trick exp on DVE:
#   P = bitcast_bf16(int16(A*x + 127*128 + sigma)) ~= e^x
SCH_A = 128.0 / math.log(2.0)
SCH_SIGMA = -7.0
SCH_B = 127.0 * 128.0 + SCH_SIGMA
N_WARMUP = 56  # dummy PE transposes to hold the HAM clock gate open


def _build():
    import concourse.bass as bass  # noqa: F401
    import concourse.mybir as mybir
    import concourse.tile as tile
    from concourse import bacc

    f32 = mybir.dt.float32
    f32r = mybir.dt.float32r
    i32 = mybir.dt.int32
    i16 = mybir.dt.int16
    bf16 = mybir.dt.bfloat16
    Exp = mybir.ActivationFunctionType.Exp
    Cp = mybir.ActivationFunctionType.Copy
    Alu = mybir.AluOpType

    nc = bacc.Bacc("TRN2", target_bir_lowering=False, debug=False, num_devices=NCORES)

    q_d = nc.dram_tensor("q_s", [LQ, H], f32, kind="ExternalInput")
    k_d = nc.dram_tensor("k_s", [L, H], f32, kind="ExternalInput")
    v_d = nc.dram_tensor("v_s", [L, H], f32, kind="ExternalInput")
    kb_d = nc.dram_tensor("kb_s", [L, H], f32, kind="ExternalInput")
    mask_d = nc.dram_tensor("mask_s", [L], i32, kind="ExternalInput")
    sw_d = nc.dram_tensor("sw_s", [NH, LQ], f32, kind="ExternalInput")
    Wb_d = nc.dram_tensor("Wb", [H, H], f32, kind="ExternalInput")
    bb_d = nc.dram_tensor("bb", [H], f32, kind="ExternalInput")
    Ww_d = nc.dram_tensor("Ww", [H, H], f32, kind="ExternalInput")
    bw_d = nc.dram_tensor("bw", [H], f32, kind="ExternalInput")
    id_d = nc.dram_tensor("ident", [128, 128], f32, kind="ExternalInput")
    out_d = nc.dram_tensor("out", [LQ, H], f32, kind="ExternalOutput")
    DBG = False
    if DBG:
        dbg_keff = [nc.dram_tensor(f"dbg_keff{g}", [128, L], f32,
                                   kind="ExternalOutput") for g in range(2)]
        dbg_qs = [nc.dram_tensor(f"dbg_qs{g}", [128, LQ], f32,
                                 kind="ExternalOutput") for g in range(2)]
        dbg_vmm = nc.dram_tensor("dbg_vmm", [128, 16 * NH * 64], bf16,
                                 kind="ExternalOutput")
        dbg_hid = [nc.dram_tensor(f"dbg_hid{g}", [128, LQ], f32,
                                  kind="ExternalOutput") for g in range(2)]

    copy_flip = [0]

    with tile.TileContext(nc) as tc:
        with (
            tc.tile_pool(name="persist", bufs=1) as pp,
            tc.tile_pool(name="pt", bufs=7) as ptp,
            tc.tile_pool(name="small", bufs=4) as smp,
        ):
            # ---------------- persistent SBUF tensors ----------------
            ident = pp.tile([128, 128], f32, tag="ident")
            nc.sync.dma_start(out=ident, in_=id_d[:, :])
            keffT = [pp.tile([128, L], bf16, tag=f"keffT{g}", name=f"keffT{g}")
                     for g in range(2)]
            qsT = [pp.tile([128, LQ], bf16, tag=f"qsT{g}", name=f"qsT{g}")
                   for g in range(2)]
            # per (key-chunk, head): [v_hi | mask] -> [128, 64] bf16
            vmm = pp.tile([128, 16 * NH * 64], bf16, tag="vmm")
            hidT = [pp.tile([128, LQ], bf16, tag=f"hidT{g}", name=f"hidT{g}")
                    for g in range(2)]
            WwT = [pp.tile([128, H], bf16, tag=f"WwT{g}", name=f"WwT{g}")
                   for g in range(2)]
            ones1 = pp.tile([1, 128], f32, tag="ones1")
            nc.vector.memset(ones1, 1.0)
            ones1r = pp.tile([1, 128], f32r, tag="ones1r")
            nc.vector.tensor_copy(ones1r, ones1)
            bwr = pp.tile([1, H], f32r, tag="bwr")
            sc8 = pp.tile([128, 64], f32, tag="sc8")
            outsb = pp.tile([128, 8 * H], f32, tag="outsb")
            bbr = pp.tile([1, H], bf16, tag="bbr")
            oneslr = pp.tile([1, L], bf16, tag="oneslr")
            nc.vector.memset(oneslr, 1.0)

            with tc.tile_pool(name="stage", bufs=1) as sp:
                def pcopy(dst, src):
                    # alternate psum->sbuf evacuation between DVE and ACT
                    if copy_flip[0] % 2 == 0:
                        nc.vector.tensor_copy(dst, src)
                    else:
                        nc.scalar.copy(dst, src)
                    copy_flip[0] += 1

                # warm the ACT exp table before the main loop needs it
                dummy = sp.tile([1, 128], f32, tag="dummy")
                nc.vector.memset(dummy, 0.0)
                dummy2 = sp.tile([1, 128], f32, tag="dummy2")
                nc.scalar.activation(dummy2, dummy, Exp)

                # ---------------- staging loads (critical path first) ----
                m16 = sp.tile([16, 128], i32, tag="m16")
                nc.sync.dma_start(out=m16,
                                  in_=mask_d.rearrange("(c p) -> c p", p=128))
                swt = sp.tile([NH, LQ], f32, tag="swt")
                nc.sync.dma_start(out=swt, in_=sw_d[:, :])
                wbraw = sp.tile([128, 2 * H], f32, tag="wbraw")
                nc.sync.dma_start(out=wbraw.rearrange("p (c e) -> p c e", c=2),
                                  in_=Wb_d.rearrange("(c p) e -> p c e", p=128))
                qraw = sp.tile([128, 8 * H], f32, tag="qraw")
                nc.sync.dma_start(out=qraw.rearrange("p (c e) -> p c e", c=8),
                                  in_=q_d.rearrange("(c p) e -> p c e", p=128))
                bbt = sp.tile([1, H], f32, tag="bbt")
                nc.sync.dma_start(out=bbt, in_=bb_d[None, :])
                nc.vector.tensor_copy(bbr, bbt)
                kbraw = sp.tile([128, 16 * H], f32, tag="kbraw")
                kraw = sp.tile([128, 16 * H], f32, tag="kraw")
                vraw = sp.tile([128, 16 * H], f32, tag="vraw")

                def load4(tile_, dram, c4, eng=None):
                    tv = tile_.rearrange("p (c e) -> p c e", c=16)
                    dv = dram.rearrange("(c p) e -> p c e", p=128)
                    (eng or nc.sync).dma_start(
                        out=tv[:, c4 * 4:(c4 + 1) * 4, :],
                        in_=dv[:, c4 * 4:(c4 + 1) * 4, :])

                for c4 in range(4):
                    load4(kraw, k_d, c4, eng=nc.scalar)
                for c4 in range(4):
                    load4(kbraw, kb_d, c4)
                for c4 in range(4):
                    load4(vraw, v_d, c4, eng=nc.scalar)
                wwraw = sp.tile([128, 2 * H], f32, tag="wwraw")
                nc.scalar.dma_start(out=wwraw.rearrange("p (c e) -> p c e", c=2),
                                    in_=Ww_d.rearrange("(c p) e -> p c e", p=128))
                bwt = sp.tile([1, H], f32, tag="bwt")
                nc.scalar.dma_start(out=bwt, in_=bw_d[None, :])
                nc.vector.tensor_copy(bwr, bwt)
                m16f = sp.tile([16, 128], f32, tag="m16f")
                nc.vector.tensor_copy(m16f, m16)
                maskf = sp.tile([128, 16], f32, tag="maskf")
                WbT = [sp.tile([128, H], bf16, tag=f"WbT{e}", name=f"WbT{e}")
                       for e in range(2)]
                kbT = [sp.tile([128, L], bf16, tag=f"kbT{e}", name=f"kbT{e}")
                       for e in range(2)]

                # ---------------- prep: transposes & keff ----------------
                with (
                    tc.tile_pool(name="ptr", bufs=4, space="PSUM") as ptr,
                    tc.tile_pool(name="pkeff", bufs=1, space="PSUM") as pkf,
                ):
                    # PE warmup: keep the HAM activity monitor busy during
                    # the DMA head so the clock gate opens (and stays open)
                    # before the real transpose burst.
                    for w in range(N_WARMUP):
                        t = ptr.tile([128, 128], f32, tag="tr", name="warm")
                        nc.tensor.transpose(t, ident, ident)

                    # mask -> maskf [128, 16]
                    tm = ptr.tile([128, 16], f32, tag="tr", name="tm")
                    nc.tensor.transpose(tm, m16f, ident[0:16, 0:16])
                    nc.vector.tensor_copy(maskf, tm)

                    # scale_w slices -> sc8 [128, 8 per q-chunk]
                    for mq in range(8):
                        t = ptr.tile([128, 8], f32, tag="tr", name="t")
                        nc.tensor.transpose(t, swt[:, mq * 128:(mq + 1) * 128],
                                            ident[0:NH, 0:NH])
                        nc.vector.tensor_copy(sc8[:, mq * 8:(mq + 1) * 8], t)

                    # Wb transposes -> WbT bf16
                    for dc in range(2):
                        for ec in range(2):
                            t = ptr.tile([128, 128], f32, tag="tr", name="t")
                            nc.tensor.transpose(
                                t,
                                wbraw[:, dc * H + ec * 128: dc * H + (ec + 1) * 128],
                                ident)
                            pcopy(WbT[ec][:, dc * 128:(dc + 1) * 128], t)

                    # k_b transpose -> kbT bf16 (4 transposes per
                    # psum tile, one wide evacuation copy each)
                    for lb in range(4):
                        for ec in range(2):
                            tb = ptr.tile([128, 512], f32, tag="tr",
                                          name="tb")
                            for j in range(4):
                                lc = lb * 4 + j
                                nc.tensor.transpose(
                                    tb[:, j * 128:(j + 1) * 128],
                                    kbraw[:, lc * H + ec * 128:
                                          lc * H + (ec + 1) * 128],
                                    ident)
                            pcopy(kbT[ec][:, lb * 512:(lb + 1) * 512], tb)

                    # q: scale by scale_w/sqrt(DH) (DVE, in place)
                    for mq in range(8):
                        qv = qraw[:, mq * H:(mq + 1) * H].rearrange(
                            "p (h j) -> p h j", h=NH)
                        nc.vector.scalar_tensor_tensor(
                            out=qv, in0=qv, scalar=ISQ,
                            in1=sc8[:, mq * 8:(mq + 1) * 8][:, :, None].broadcast_to(
                                [128, 8, 32]),
                            op0=Alu.mult, op1=Alu.mult)

                    def keff_mms_first(dc, pk):
                        for ns in range(4):
                            for ec in range(2):
                                nc.tensor.matmul(
                                    pk[:, ns * 512:(ns + 1) * 512],
                                    lhsT=WbT[ec][:, dc * 128:(dc + 1) * 128],
                                    rhs=kbT[ec][:, ns * 512:(ns + 1) * 512],
                                    start=(ec == 0), stop=False)
                            nc.tensor.matmul(
                                pk[:, ns * 512:(ns + 1) * 512],
                                lhsT=bbr[0:1, dc * 128:(dc + 1) * 128],
                                rhs=oneslr[0:1, ns * 512:(ns + 1) * 512],
                                start=False, stop=False)

                    def keff_transposes(dc, pk):
                        # accumulate k^T on top of kb@Wb^T + bb; per-bank
                        # stop + immediate evacuation
                        for ns in range(4):
                            for j in range(4):
                                lc = ns * 4 + j
                                nc.tensor.matmul(
                                    pk[:, lc * 128:(lc + 1) * 128],
                                    lhsT=kraw[:, lc * H + dc * 128:
                                              lc * H + (dc + 1) * 128],
                                    rhs=ident,
                                    is_transpose=True,
                                    start=False, stop=(j == 3))
                            pcopy(keffT[dc][:, ns * 512:(ns + 1) * 512],
                                  pk[:, ns * 512:(ns + 1) * 512])

                    def warm(n):
                        for _ in range(n):
                            t = ptr.tile([128, 128], f32, tag="tr",
                                         name="warm")
                            nc.tensor.transpose(t, ident, ident)

                    def qtrans(mqs):
                        mqs = list(mqs)
                        for dc in range(2):
                            for b in range(0, len(mqs), 4):
                                blk = mqs[b:b + 4]
                                tb = ptr.tile([128, 512], f32, tag="tr",
                                              name="tb")
                                for j, mq in enumerate(blk):
                                    nc.tensor.transpose(
                                        tb[:, j * 128:(j + 1) * 128],
                                        qraw[:, mq * H + dc * 128:
                                             mq * H + (dc + 1) * 128],
                                        ident)
                                pcopy(qsT[dc][:, blk[0] * 128:
                                              (blk[-1] + 1) * 128], tb)

                    warm(12)
                    pk0 = pkf.tile([128, L], f32, tag="pk", name="pk0")
                    keff_mms_first(0, pk0)
                    warm(8)
                    keff_transposes(0, pk0)

                    qtrans(range(4))
                    pk1 = pkf.tile([128, L], f32, tag="pk", name="pk1")
                    keff_mms_first(1, pk1)
                    warm(6)
                    keff_transposes(1, pk1)

                    # tail PE work (not needed until later in the main
                    # loop) keeps HAM warm through the psum-bank handoff
                    qtrans(range(4, 8))
                    for er in range(2):
                        for g in range(2):
                            t = ptr.tile([128, 128], f32, tag="tr", name="t")
                            nc.tensor.transpose(
                                t,
                                wwraw[:, er * H + g * 128: er * H + (g + 1) * 128],
                                ident)
                            pcopy(WwT[g][:, er * 128:(er + 1) * 128], t)
                    warm(8)

                    # vmm build late so ACT's critical pcopies are not
                    # queued behind it; mask-reps on the idle Pool engine
                    vmm4 = vmm.rearrange("p (c h w) -> p c h w", c=16, h=NH)
                    vraw3 = vraw.rearrange("p (c e) -> p c e", c=16)
                    for lc in range(16):
                        vsl = vraw3[:, lc, :].rearrange("p (h j) -> p h j", h=NH)
                        nc.scalar.activation(vmm4[:, lc, :, 0:32], vsl, Cp,
                                             scale=maskf[:, lc:lc + 1])
                        nc.gpsimd.tensor_copy(
                            vmm4[:, lc, :, 32:64],
                            maskf[:, lc:lc + 1][:, :, None].broadcast_to(
                                [128, NH, 32]))

            # ---------------- main attention loop ----------------
            # group (g, qb): heads (2g, 2g+1), queries qb*512..+512
            # per kc: [PV(kc-3); S(kc); exp(kc)] depth-3 pipeline.
            # exp engine alternates by kc parity: even=ACT exp, odd=DVE
            # Schraudolph. Normalization of the previous group's pv is
            # emitted into the DVE-idle even-kc slots.
            with (
                tc.tile_pool(name="pst", bufs=3, space="PSUM") as pst,
                tc.tile_pool(name="ppv", bufs=2, space="PSUM") as ppv,
            ):
                groups = [(g, qb) for g in range(4) for qb in range(2)]
                prev_norm = []

                for g, qb in groups:
                    ch = g // 2
                    pv = ppv.tile([128, 512], f32, tag="pv", name=f"pv{g}_{qb}")
                    pts = {}

                    def emit_pv(kc, pv=pv, pts=pts, g=g):
                        for t in range(2):
                            h = 2 * g + t
                            nc.tensor.matmul(
                                pv[64 * t:64 * t + 64, :],
                                lhsT=vmm[:, (kc * NH + h) * 64:
                                         (kc * NH + h) * 64 + 64],
                                rhs=pts[kc][:, t * 512:(t + 1) * 512],
                                tile_position=(0, 64 * t),
                                start=(kc == 0), stop=(kc == 15),
                                skip_group_check=True)

                    for kc in range(16):
                        st = pst.tile([128, 1024], f32, tag="st", name="st")
                        for t in range(2):
                            ro = (g % 2) * 64 + t * 32
                            nc.tensor.matmul(
                                st[:, t * 512:(t + 1) * 512],
                                lhsT=keffT[ch][ro:ro + 32,
                                               kc * 128:(kc + 1) * 128],
                                rhs=qsT[ch][ro:ro + 32,
                                            qb * 512:(qb + 1) * 512],
                                tile_position=(ro, 0),
                                start=True, stop=True)
                        pt = ptp.tile([128, 1024], bf16, tag="pt", name="pt")
                        if kc % 2 == 0:
                            nc.scalar.activation(pt, st, Exp)
                        else:
                            nc.vector.tensor_scalar(
                                out=pt.bitcast(i16), in0=st,
                                scalar1=SCH_A, scalar2=SCH_B,
                                op0=Alu.mult, op1=Alu.add)
                        pts[kc] = pt
                        if kc >= 5:
                            emit_pv(kc - 5)
                        # previous group's normalize in DVE-idle even slots
                        if prev_norm and kc in (1, 3, 5, 7):
                            prev_norm[kc // 2]()
                    for kc in (11, 12, 13, 14, 15):
                        emit_pv(kc)

                    def make_norm(pv=pv, g=g, qb=qb, ch=ch):
                        # reciprocal_approx_fast misreads PSUM operands on
                        # HW (sim-only correct) — stage the denominator
                        # through SBUF via an ACT copy first.
                        state = {}
                        pieces = []
                        for t in range(2):
                            ro = (g % 2) * 64 + t * 32

                            def piece_a(pv=pv, t=t, state=state):
                                rs = smp.tile([32, 512], f32, tag="rsum",
                                              name="rsum")
                                nc.scalar.copy(
                                    rs, pv[64 * t + 32:64 * t + 64, :])
                                r = smp.tile([32, 512], f32, tag="rcp",
                                             name="rcp")
                                nc.vector.reciprocal_approx_fast(r, rs)
                                state[t] = r

                            def piece_b(pv=pv, t=t, ro=ro, ch=ch, qb=qb,
                                        state=state):
                                nc.vector.tensor_mul(
                                    hidT[ch][ro:ro + 32,
                                             qb * 512:(qb + 1) * 512],
                                    pv[64 * t:64 * t + 32, :], state[t])

                            pieces.append(piece_a)
                            pieces.append(piece_b)
                        return pieces

                    prev_norm = make_norm()

                # final group's normalize
                for p in prev_norm:
                    p()

            if DBG:
                for g in range(2):
                    nc.sync.dma_start(out=dbg_keff[g][:, :],
                                      in_=keffT[g].bitcast(f32))
                    nc.sync.dma_start(out=dbg_qs[g][:, :],
                                      in_=qsT[g].bitcast(f32))
                    nc.sync.dma_start(out=dbg_hid[g][:, :],
                                      in_=hidT[g].bitcast(f32))
                nc.sync.dma_start(out=dbg_vmm[:, :], in_=vmm)

            # ---------------- output linear ----------------
            with tc.tile_pool(name="pout", bufs=2, space="PSUM") as pout:
                for mq in range(8):
                    po = pout.tile([128, H], f32, tag="po", name="po")
                    for g in range(2):
                        nc.tensor.matmul(
                            po,
                            lhsT=hidT[g][:, mq * 128:(mq + 1) * 128],
                            rhs=WwT[g],
                            start=(g == 0), stop=False)
                    nc.tensor.matmul(
                        po, lhsT=ones1r, rhs=bwr, start=False, stop=True)
                    pcopy(outsb[:, mq * H:(mq + 1) * H], po)
                    if mq % 2 == 1:
                        nc.sync.dma_start(
                            out=out_d.rearrange("(c p) e -> p c e", p=128)[
                                :, mq - 1:mq + 1, :],
                            in_=outsb.rearrange("p (c e) -> p c e", c=8)[
                                :, mq - 1:mq + 1, :])

    nc.compile()
    return nc


def _make_in_maps(inputs):
    q = np.ascontiguousarray(np.asarray(inputs["q"], dtype=np.float32))
    k = np.ascontiguousarray(np.asarray(inputs["k"], dtype=np.float32))
    v = np.ascontiguousarray(np.asarray(inputs["v"], dtype=np.float32))
    k_b = np.ascontiguousarray(np.asarray(inputs["k_b"], dtype=np.float32))
    mask = np.ascontiguousarray(np.asarray(inputs["mask"], dtype=np.int32))
    sw = np.ascontiguousarray(np.asarray(inputs["scale_w"], dtype=np.float32))
    Wb = np.ascontiguousarray(np.asarray(inputs["Wb"], dtype=np.float32))
    bb = np.ascontiguousarray(np.asarray(inputs["bb"], dtype=np.float32))
    Ww = np.ascontiguousarray(np.asarray(inputs["Ww"], dtype=np.float32))
    bw = np.ascontiguousarray(np.asarray(inputs["bw"], dtype=np.float32))
    ident = np.eye(128, dtype=np.float32)
    in_maps = []
    for c in range(NCORES):
        b, qs = c // 2, c % 2
        in_maps.append({
            "q_s": q[b, qs * LQ:(qs + 1) * LQ, :],
            "k_s": k[b],
            "v_s": v[b],
            "kb_s": k_b[b],
            "mask_s": mask[b],
            "sw_s": np.ascontiguousarray(sw[:, qs * LQ:(qs + 1) * LQ]),
            "Wb": Wb, "bb": bb, "Ww": Ww, "bw": bw,
            "ident": ident,
        })
    return in_maps


def run_sharded(inputs, trace=False, tmpdir=None):
    from concourse.bass_utils import run_bass_kernel_spmd
    from concourse import bass_utils

    if trace:
        _install_ntff_hook()
        bass_utils.upload_artifacts = lambda d: d
    nc = _build()
    in_maps = _make_in_maps(inputs)
    res = run_bass_kernel_spmd(nc, in_maps, list(range(NCORES)),
                               trace=trace, tmpdir=tmpdir)
    out = np.empty((B, L, H), dtype=np.float32)
    for c in range(NCORES):
        b, qs = c // 2, c % 2
        out[b, qs * LQ:(qs + 1) * LQ, :] = res.results[c]["out"]
    return out, res


def kernel(**inputs):
    out, _ = run_sharded(inputs, trace=False)
    return out


def _install_ntff_hook():
    """Provide antenv.axon_hooks (absent in this image) so trace=True works."""
    import contextlib
    import ctypes
    import types

    import antenv

    if hasattr(antenv, "axon_hooks"):
        return
    mod = types.ModuleType("antenv.axon_hooks")
    _hook = [None]
    mod.set_axon_ntff_profile_hook = lambda h: _hook.__setitem__(0, h)
    mod.get_axon_ntff_profile_hook = lambda: _hook[0]
    antenv.axon_hooks = mod
    sys.modules["antenv.axon_hooks"] = mod

    lib = ctypes.CDLL("/opt/axon/libaxon_pjrt.so")
    if not hasattr(lib, "axon_start_nrt_profile"):
        return
    lib.axon_start_nrt_profile.argtypes = [ctypes.POINTER(ctypes.c_int64),
                                           ctypes.c_size_t]
    lib.axon_start_nrt_profile.restype = ctypes.c_int64
    lib.axon_stop_nrt_profile.argtypes = [ctypes.c_char_p]
    lib.axon_stop_nrt_profile.restype = ctypes.c_int64

    @contextlib.contextmanager
    def _profile(output_dir, device_ids):
        import jax

        jax.devices()
        if device_ids:
            ids = (ctypes.c_int64 * len(device_ids))(*device_ids)
            rc = lib.axon_start_nrt_profile(ids, len(device_ids))
        else:
            rc = lib.axon_start_nrt_profile(None, 0)
        if rc != 0:
            raise RuntimeError(f"axon_start_nrt_profile rc={rc}")
        try:
            yield
        finally:
            n = lib.axon_stop_nrt_profile(str(output_dir).encode())
            print(f"profile: {n} file(s) written to {output_dir}",
                  file=sys.stderr)

    mod.set_axon_ntff_profile_hook(_profile)


# revision 44
# speedup vs baseline: 1.0275x; 1.0144x over previous
"""Trainium2 Bass kernel for AuxiliaryMultiHeadedAttention (v2).

Reference computation (B=4, L=2048, H=256, NH=8, DH=32):
    kb   = split_heads(k_b @ Wb.T + bb)
    corr = (qh @ kh^T + qh @ kb^T) / sqrt(DH) * scale_w[h, q]
    corr = where(mask==0, -1e9, corr);  prob = softmax(corr)
    out  = merge_heads(prob @ vh) @ Ww.T + bw

Kernel strategy (8 NeuronCores):
    Shard (batch, query-half): core c -> batch c//2, queries (c%2)*1024..+1024.

    v2 changes vs v1 (253.8us -> ~215us):
      * Main-loop S matmuls in bf16 (keffT/qsT/WbT/kbT bf16): same
        1 cyc/row as f32r, faster LDWEIGHTS, lower PE power.
      * exp split across engines by kc parity: even kc tiles on ACT (true
        exp), odd kc tiles on DVE via a Schraudolph bit-trick
        (int16(184.66*x + 16249) bitcast to bf16 ~= e^x, sigma=-7 tuned so
        the estimator is unbiased vs ACT's exp — the bias must cancel
        between the interleaved key chunks or softmax mass shifts).
      * Depth-5 software pipeline [S(kc); exp(kc); PV(kc-5)] (st tiles are
        freed by exp, not PV, so PV depth is free in PSUM) reduces
        exp->PV stalls that re-trip the HAM clock gate to K=4/8 (1.2 GHz).
      * PSUM: st bufs=3 (2 banks each) + pv bufs=2 (1 bank) = 8 banks.
      * normalize: denominator copied PSUM->SBUF on ACT first
        (reciprocal_approx_fast misreads PSUM operands on HW, sim-only
        correct!), reciprocal+mul on DVE in the next group's idle slots.
      * PE kept busy through prep DMA waits and the prep->main PSUM bank
        handoff with dummy ident transposes (HAM warmup); input DMAs
        split across SP and ACT queues; per-bank keff evacuation.
"""

import sys

if "/opt/trn_rl_repo" not in sys.path:
    sys.path.insert(0, "/opt/trn_rl_repo")

import math

import numpy as np

B, L, H, NH, DH = 4, 2048, 256, 8, 32
LQ = 1024  # queries per core
NCORES = 8
ISQ = 1.0 / math.sqrt(DH)
# Schraudolph constants for bf16 bit-trick exp on DVE:
#   P = bitcast_bf16(int16(A*x + 127*128 + sigma)) ~= e^x
SCH_A = 128.0 / math.log(2.0)
SCH_SIGMA = -7.0
SCH_B = 127.0 * 128.0 + SCH_SIGMA
N_WARMUP = 56  # dummy PE transposes to hold the HAM clock gate open


def _build():
    import concourse.bass as bass  # noqa: F401
    import concourse.mybir as mybir
    import concourse.tile as tile
    from concourse import bacc

    f32 = mybir.dt.float32
    f32r = mybir.dt.float32r
    i32 = mybir.dt.int32
    i16 = mybir.dt.int16
    bf16 = mybir.dt.bfloat16
    Exp = mybir.ActivationFunctionType.Exp
    Cp = mybir.ActivationFunctionType.Copy
    Alu = mybir.AluOpType

    nc = bacc.Bacc("TRN2", target_bir_lowering=False, debug=False, num_devices=NCORES)

    q_d = nc.dram_tensor("q_s", [LQ, H], f32, kind="ExternalInput")
    k_d = nc.dram_tensor("k_s", [L, H], f32, kind="ExternalInput")
    v_d = nc.dram_tensor("v_s", [L, H], f32, kind="ExternalInput")
    kb_d = nc.dram_tensor("kb_s", [L, H], f32, kind="ExternalInput")
    mask_d = nc.dram_tensor("mask_s", [L], i32, kind="ExternalInput")
    sw_d = nc.dram_tensor("sw_s", [NH, LQ], f32, kind="ExternalInput")
    Wb_d = nc.dram_tensor("Wb", [H, H], f32, kind="ExternalInput")
    bb_d = nc.dram_tensor("bb", [H], f32, kind="ExternalInput")
    Ww_d = nc.dram_tensor("Ww", [H, H], f32, kind="ExternalInput")
    bw_d = nc.dram_tensor("bw", [H], f32, kind="ExternalInput")
    id_d = nc.dram_tensor("ident", [128, 128], f32, kind="ExternalInput")
    out_d = nc.dram_tensor("out", [LQ, H], f32, kind="ExternalOutput")
    DBG = False
    if DBG:
        dbg_keff = [nc.dram_tensor(f"dbg_keff{g}", [128, L], f32,
                                   kind="ExternalOutput") for g in range(2)]
        dbg_qs = [nc.dram_tensor(f"dbg_qs{g}", [128, LQ], f32,
                                 kind="ExternalOutput") for g in range(2)]
        dbg_vmm = nc.dram_tensor("dbg_vmm", [128, 16 * NH * 64], bf16,
                                 kind="ExternalOutput")
        dbg_hid = [nc.dram_tensor(f"dbg_hid{g}", [128, LQ], f32,
                                  kind="ExternalOutput") for g in range(2)]

    copy_flip = [0]

    with tile.TileContext(nc) as tc:
        with (
            tc.tile_pool(name="persist", bufs=1) as pp,
            tc.tile_pool(name="pt", bufs=7) as ptp,
            tc.tile_pool(name="small", bufs=4) as smp,
        ):
            # ---------------- persistent SBUF tensors ----------------
            ident = pp.tile([128, 128], f32, tag="ident")
            nc.sync.dma_start(out=ident, in_=id_d[:, :])
            keffT = [pp.tile([128, L], bf16, tag=f"keffT{g}", name=f"keffT{g}")
                     for g in range(2)]
            qsT = [pp.tile([128, LQ], bf16, tag=f"qsT{g}", name=f"qsT{g}")
                   for g in range(2)]
            # per (key-chunk, head): [v_hi | mask] -> [128, 64] bf16
            vmm = pp.tile([128, 16 * NH * 64], bf16, tag="vmm")
            hidT = [pp.tile([128, LQ], bf16, tag=f"hidT{g}", name=f"hidT{g}")
                    for g in range(2)]
            WwT = [pp.tile([128, H], bf16, tag=f"WwT{g}", name=f"WwT{g}")
                   for g in range(2)]
            ones1 = pp.tile([1, 128], f32, tag="ones1")
            nc.vector.memset(ones1, 1.0)
            ones1r = pp.tile([1, 128], f32r, tag="ones1r")
            nc.vector.tensor_copy(ones1r, ones1)
            bwr = pp.tile([1, H], f32r, tag="bwr")
            sc8 = pp.tile([128, 64], f32, tag="sc8")
            outsb = pp.tile([128, 8 * H], f32, tag="outsb")
            bbr = pp.tile([1, H], bf16, tag="bbr")
            oneslr = pp.tile([1, L], bf16, tag="oneslr")
            nc.vector.memset(oneslr, 1.0)

            with tc.tile_pool(name="stage", bufs=1) as sp:
                def pcopy(dst, src):
                    # alternate psum->sbuf evacuation between DVE and ACT
                    if copy_flip[0] % 2 == 0:
                        nc.vector.tensor_copy(dst, src)
                    else:
                        nc.scalar.copy(dst, src)
                    copy_flip[0] += 1

                # warm the ACT exp table before the main loop needs it
                dummy = sp.tile([1, 128], f32, tag="dummy")
                nc.vector.memset(dummy, 0.0)
                dummy2 = sp.tile([1, 128], f32, tag="dummy2")
                nc.scalar.activation(dummy2, dummy, Exp)

                # ---------------- staging loads (critical path first) ----
                m16 = sp.tile([16, 128], i32, tag="m16")
                nc.sync.dma_start(out=m16,
                                  in_=mask_d.rearrange("(c p) -> c p", p=128))
                swt = sp.tile([NH, LQ], f32, tag="swt")
                nc.sync.dma_start(out=swt, in_=sw_d[:, :])
                wbraw = sp.tile([128, 2 * H], f32, tag="wbraw")
                nc.sync.dma_start(out=wbraw.rearrange("p (c e) -> p c e", c=2),
                                  in_=Wb_d.rearrange("(c p) e -> p c e", p=128))
                qraw = sp.tile([128, 8 * H], f32, tag="qraw")
                nc.sync.dma_start(out=qraw.rearrange("p (c e) -> p c e", c=8),
                                  in_=q_d.rearrange("(c p) e -> p c e", p=128))
                bbt = sp.tile([1, H], f32, tag="bbt")
                nc.sync.dma_start(out=bbt, in_=bb_d[None, :])
                nc.vector.tensor_copy(bbr, bbt)
                kbraw = sp.tile([128, 16 * H], f32, tag="kbraw")
                kraw = sp.tile([128, 16 * H], f32, tag="kraw")
                vraw = sp.tile([128, 16 * H], f32, tag="vraw")

                def load4(tile_, dram, c4, eng=None):
                    tv = tile_.rearrange("p (c e) -> p c e", c=16)
                    dv = dram.rearrange("(c p) e -> p c e", p=128)
                    (eng or nc.sync).dma_start(
                        out=tv[:, c4 * 4:(c4 + 1) * 4, :],
                        in_=dv[:, c4 * 4:(c4 + 1) * 4, :])

                for c4 in range(4):
                    load4(kraw, k_d, c4, eng=nc.scalar)
                for c4 in range(4):
                    load4(kbraw, kb_d, c4)
                for c4 in range(4):
                    load4(vraw, v_d, c4, eng=nc.scalar)
                wwraw = sp.tile([128, 2 * H], f32, tag="wwraw")
                nc.scalar.dma_start(out=wwraw.rearrange("p (c e) -> p c e", c=2),
                                    in_=Ww_d.rearrange("(c p) e -> p c e", p=128))
                bwt = sp.tile([1, H], f32, tag="bwt")
                nc.scalar.dma_start(out=bwt, in_=bw_d[None, :])
                nc.vector.tensor_copy(bwr, bwt)
                m16f = sp.tile([16, 128], f32, tag="m16f")
                nc.vector.tensor_copy(m16f, m16)
                maskf = sp.tile([128, 16], f32, tag="maskf")
                WbT = [sp.tile([128, H], bf16, tag=f"WbT{e}", name=f"WbT{e}")
                       for e in range(2)]
                kbT = [sp.tile([128, L], bf16, tag=f"kbT{e}", name=f"kbT{e}")
                       for e in range(2)]

                # ---------------- prep: transposes & keff ----------------
                with (
                    tc.tile_pool(name="ptr", bufs=4, space="PSUM") as ptr,
                    tc.tile_pool(name="pkeff", bufs=1, space="PSUM") as pkf,
                ):
                    # PE warmup: keep the HAM activity monitor busy during
                    # the DMA head so the clock gate opens (and stays open)
                    # before the real transpose burst.
                    for w in range(N_WARMUP):
                        t = ptr.tile([128, 128], f32, tag="tr", name="warm")
                        nc.tensor.transpose(t, ident, ident)

                    # mask -> maskf [128, 16]
                    tm = ptr.tile([128, 16], f32, tag="tr", name="tm")
                    nc.tensor.transpose(tm, m16f, ident[0:16, 0:16])
                    nc.vector.tensor_copy(maskf, tm)

                    # scale_w slices -> sc8 [128, 8 per q-chunk]
                    for mq in range(8):
                        t = ptr.tile([128, 8], f32, tag="tr", name="t")
                        nc.tensor.transpose(t, swt[:, mq * 128:(mq + 1) * 128],
                                            ident[0:NH, 0:NH])
                        nc.vector.tensor_copy(sc8[:, mq * 8:(mq + 1) * 8], t)

                    # Wb transposes -> WbT bf16
                    for dc in range(2):
                        for ec in range(2):
                            t = ptr.tile([128, 128], f32, tag="tr", name="t")
                            nc.tensor.transpose(
                                t,
                                wbraw[:, dc * H + ec * 128: dc * H + (ec + 1) * 128],
                                ident)
                            pcopy(WbT[ec][:, dc * 128:(dc + 1) * 128], t)

                    # k_b transpose -> kbT bf16 (4 transposes per
                    # psum tile, one wide evacuation copy each)
                    for lb in range(4):
                        for ec in range(2):
                            tb = ptr.tile([128, 512], f32, tag="tr",
                                          name="tb")
                            for j in range(4):
                                lc = lb * 4 + j
                                nc.tensor.transpose(
                                    tb[:, j * 128:(j + 1) * 128],
                                    kbraw[:, lc * H + ec * 128:
                                          lc * H + (ec + 1) * 128],
                                    ident)
                            pcopy(kbT[ec][:, lb * 512:(lb + 1) * 512], tb)

                    # q: scale by scale_w/sqrt(DH) (DVE, in place)
                    for mq in range(8):
                        qv = qraw[:, mq * H:(mq + 1) * H].rearrange(
                            "p (h j) -> p h j", h=NH)
                        nc.vector.scalar_tensor_tensor(
                            out=qv, in0=qv, scalar=ISQ,
                            in1=sc8[:, mq * 8:(mq + 1) * 8][:, :, None].broadcast_to(
                                [128, 8, 32]),
                            op0=Alu.mult, op1=Alu.mult)

                    def keff_mms_first(dc, pk):
                        for ns in range(4):
                            for ec in range(2):
                                nc.tensor.matmul(
                                    pk[:, ns * 512:(ns + 1) * 512],
                                    lhsT=WbT[ec][:, dc * 128:(dc + 1) * 128],
                                    rhs=kbT[ec][:, ns * 512:(ns + 1) * 512],
                                    start=(ec == 0), stop=False)
                            nc.tensor.matmul(
                                pk[:, ns * 512:(ns + 1) * 512],
                                lhsT=bbr[0:1, dc * 128:(dc + 1) * 128],
                                rhs=oneslr[0:1, ns * 512:(ns + 1) * 512],
                                start=False, stop=False)

                    def keff_transposes(dc, pk):
                        # accumulate k^T on top of kb@Wb^T + bb; per-bank
                        # stop + immediate evacuation
                        for ns in range(4):
                            for j in range(4):
                                lc = ns * 4 + j
                                nc.tensor.matmul(
                                    pk[:, lc * 128:(lc + 1) * 128],
                                    lhsT=kraw[:, lc * H + dc * 128:
                                              lc * H + (dc + 1) * 128],
                                    rhs=ident,
                                    is_transpose=True,
                                    start=False, stop=(j == 3))
                            pcopy(keffT[dc][:, ns * 512:(ns + 1) * 512],
                                  pk[:, ns * 512:(ns + 1) * 512])

                    def warm(n):
                        for _ in range(n):
                            t = ptr.tile([128, 128], f32, tag="tr",
                                         name="warm")
                            nc.tensor.transpose(t, ident, ident)

                    def qtrans(mqs):
                        mqs = list(mqs)
                        for dc in range(2):
                            for b in range(0, len(mqs), 4):
                                blk = mqs[b:b + 4]
                                tb = ptr.tile([128, 512], f32, tag="tr",
                                              name="tb")
                                for j, mq in enumerate(blk):
                                    nc.tensor.transpose(
                                        tb[:, j * 128:(j + 1) * 128],
                                        qraw[:, mq * H + dc * 128:
                                             mq * H + (dc + 1) * 128],
                                        ident)
                                pcopy(qsT[dc][:, blk[0] * 128:
                                              (blk[-1] + 1) * 128], tb)

                    warm(12)
                    pk0 = pkf.tile([128, L], f32, tag="pk", name="pk0")
                    keff_mms_first(0, pk0)
                    warm(8)
                    keff_transposes(0, pk0)

                    qtrans(range(4))
                    pk1 = pkf.tile([128, L], f32, tag="pk", name="pk1")
                    keff_mms_first(1, pk1)
                    warm(6)
                    keff_transposes(1, pk1)

                    # tail PE work (not needed until later in the main
                    # loop) keeps HAM warm through the psum-bank handoff
                    qtrans(range(4, 8))
                    for er in range(2):
                        for g in range(2):
                            t = ptr.tile([128, 128], f32, tag="tr", name="t")
                            nc.tensor.transpose(
                                t,
                                wwraw[:, er * H + g * 128: er * H + (g + 1) * 128],
                                ident)
                            pcopy(WwT[g][:, er * 128:(er + 1) * 128], t)
                    warm(8)

                    # vmm build late so ACT's critical pcopies are not
                    # queued behind it; mask-reps on the idle Pool engine
                    vmm4 = vmm.rearrange("p (c h w) -> p c h w", c=16, h=NH)
                    vraw3 = vraw.rearrange("p (c e) -> p c e", c=16)
                    for lc in range(16):
                        vsl = vraw3[:, lc, :].rearrange("p (h j) -> p h j", h=NH)
                        nc.scalar.activation(vmm4[:, lc, :, 0:32], vsl, Cp,
                                             scale=maskf[:, lc:lc + 1])
                        nc.gpsimd.tensor_copy(
                            vmm4[:, lc, :, 32:64],
                            maskf[:, lc:lc + 1][:, :, None].broadcast_to(
                                [128, NH, 32]))

            # ---------------- main attention loop ----------------
            # group (g, qb): heads (2g, 2g+1), queries qb*512..+512
            # per kc: [PV(kc-3); S(kc); exp(kc)] depth-3 pipeline.
            # exp engine alternates by kc parity: even=ACT exp, odd=DVE
            # Schraudolph. Normalization of the previous group's pv is
            # emitted into the DVE-idle even-kc slots.
            with (
                tc.tile_pool(name="pst", bufs=3, space="PSUM") as pst,
                tc.tile_pool(name="ppv", bufs=2, space="PSUM") as ppv,
            ):
                groups = [(g, qb) for g in range(4) for qb in range(2)]
                prev_norm = []

                for g, qb in groups:
                    ch = g // 2
                    pv = ppv.tile([128, 512], f32, tag="pv", name=f"pv{g}_{qb}")
                    pts = {}

                    def emit_pv(kc, pv=pv, pts=pts, g=g):
                        for t in range(2):
                            h = 2 * g + t
                            nc.tensor.matmul(
                                pv[64 * t:64 * t + 64, :],
                                lhsT=vmm[:, (kc * NH + h) * 64:
                                         (kc * NH + h) * 64 + 64],
                                rhs=pts[kc][:, t * 512:(t + 1) * 512],
                                tile_position=(0, 64 * t),
                                start=(kc == 0), stop=(kc == 15),
                                skip_group_check=True)

                    for kc in range(16):
                        st = pst.tile([128, 1024], f32, tag="st", name="st")
                        for t in range(2):
                            ro = (g % 2) * 64 + t * 32
                            nc.tensor.matmul(
                                st[:, t * 512:(t + 1) * 512],
                                lhsT=keffT[ch][ro:ro + 32,
                                               kc * 128:(kc + 1) * 128],
                                rhs=qsT[ch][ro:ro + 32,
                                            qb * 512:(qb + 1) * 512],
                                tile_position=(ro, 0),
                                start=True, stop=True)
                        pt = ptp.tile([128, 1024], bf16, tag="pt", name="pt")
                        if kc % 2 == 0:
                            nc.scalar.activation(pt, st, Exp)
                        else:
                            nc.vector.tensor_scalar(
                                out=pt.bitcast(i16), in0=st,
                                scalar1=SCH_A, scalar2=SCH_B,
                                op0=Alu.mult, op1=Alu.add)
                        pts[kc] = pt
                        if kc >= 5:
                            emit_pv(kc - 5)
                        # previous group's normalize in DVE-idle even slots
                        if prev_norm and kc in (1, 3, 5, 7):
                            prev_norm[kc // 2]()
                    for kc in (11, 12, 13, 14, 15):
                        emit_pv(kc)

                    def make_norm(pv=pv, g=g, qb=qb, ch=ch):
                        # reciprocal_approx_fast misreads PSUM operands on
                        # HW (sim-only correct) — stage the denominator
                        # through SBUF via an ACT copy first.
                        state = {}
                        pieces = []
                        for t in range(2):
                            ro = (g % 2) * 64 + t * 32

                            def piece_a(pv=pv, t=t, state=state):
                                rs = smp.tile([32, 512], f32, tag="rsum",
                                              name="rsum")
                                nc.scalar.copy(
                                    rs, pv[64 * t + 32:64 * t + 64, :])
                                r = smp.tile([32, 512], f32, tag="rcp",
                                             name="rcp")
                                nc.vector.reciprocal_approx_fast(r, rs)
                                state[t] = r

                            def piece_b(pv=pv, t=t, ro=ro, ch=ch, qb=qb,
                                        state=state):
                                nc.vector.tensor_mul(
                                    hidT[ch][ro:ro + 32,
                                             qb * 512:(qb + 1) * 512],
                                    pv[64 * t:64 * t + 32, :], state[t])

                            pieces.append(piece_a)
                            pieces.append(piece_b)
                        return pieces

                    prev_norm = make_norm()

                # final group's normalize
                for p in prev_norm:
                    p()

            if DBG:
                for g in range(2):
                    nc.sync.dma_start(out=dbg_keff[g][:, :],
                                      in_=keffT[g].bitcast(f32))
                    nc.sync.dma_start(out=dbg_qs[g][:, :],
                                      in_=qsT[g].bitcast(f32))
                    nc.sync.dma_start(out=dbg_hid[g][:, :],
                                      in_=hidT[g].bitcast(f32))
                nc.sync.dma_start(out=dbg_vmm[:, :], in_=vmm)

            # ---------------- output linear ----------------
            with tc.tile_pool(name="pout", bufs=2, space="PSUM") as pout:
                for mq in range(8):
                    po = pout.tile([128, H], f32, tag="po", name="po")
                    for g in range(2):
                        nc.tensor.matmul(
                            po,
                            lhsT=hidT[g][:, mq * 128:(mq + 1) * 128],
                            rhs=WwT[g],
                            start=(g == 0), stop=False)
                    nc.tensor.matmul(
                        po, lhsT=ones1r, rhs=bwr, start=False, stop=True)
                    pcopy(outsb[:, mq * H:(mq + 1) * H], po)
                    if mq % 2 == 1:
                        nc.scalar.dma_start(
                            out=out_d.rearrange("(c p) e -> p c e", p=128)[
                                :, mq - 1:mq + 1, :],
                            in_=outsb.rearrange("p (c e) -> p c e", c=8)[
                                :, mq - 1:mq + 1, :])

    nc.compile()
    return nc


def _make_in_maps(inputs):
    q = np.ascontiguousarray(np.asarray(inputs["q"], dtype=np.float32))
    k = np.ascontiguousarray(np.asarray(inputs["k"], dtype=np.float32))
    v = np.ascontiguousarray(np.asarray(inputs["v"], dtype=np.float32))
    k_b = np.ascontiguousarray(np.asarray(inputs["k_b"], dtype=np.float32))
    mask = np.ascontiguousarray(np.asarray(inputs["mask"], dtype=np.int32))
    sw = np.ascontiguousarray(np.asarray(inputs["scale_w"], dtype=np.float32))
    Wb = np.ascontiguousarray(np.asarray(inputs["Wb"], dtype=np.float32))
    bb = np.ascontiguousarray(np.asarray(inputs["bb"], dtype=np.float32))
    Ww = np.ascontiguousarray(np.asarray(inputs["Ww"], dtype=np.float32))
    bw = np.ascontiguousarray(np.asarray(inputs["bw"], dtype=np.float32))
    ident = np.eye(128, dtype=np.float32)
    in_maps = []
    for c in range(NCORES):
        b, qs = c // 2, c % 2
        in_maps.append({
            "q_s": q[b, qs * LQ:(qs + 1) * LQ, :],
            "k_s": k[b],
            "v_s": v[b],
            "kb_s": k_b[b],
            "mask_s": mask[b],
            "sw_s": np.ascontiguousarray(sw[:, qs * LQ:(qs + 1) * LQ]),
            "Wb": Wb, "bb": bb, "Ww": Ww, "bw": bw,
            "ident": ident,
        })
    return in_maps


def run_sharded(inputs, trace=False, tmpdir=None):
    from concourse.bass_utils import run_bass_kernel_spmd
    from concourse import bass_utils

    if trace:
        _install_ntff_hook()
        bass_utils.upload_artifacts = lambda d: d
    nc = _build()
    in_maps = _make_in_maps(inputs)
    res = run_bass_kernel_spmd(nc, in_maps, list(range(NCORES)),
                               trace=trace, tmpdir=tmpdir)
    out = np.empty((B, L, H), dtype=np.float32)
    for c in range(NCORES):
        b, qs = c // 2, c % 2
        out[b, qs * LQ:(qs + 1) * LQ, :] = res.results[c]["out"]
    return out, res


def kernel(**inputs):
    out, _ = run_sharded(inputs, trace=False)
    return out


def _install_ntff_hook():
    """Provide antenv.axon_hooks (absent in this image) so trace=True works."""
    import contextlib
    import ctypes
    import types

    import antenv

    if hasattr(antenv, "axon_hooks"):
        return
    mod = types.ModuleType("antenv.axon_hooks")
    _hook = [None]
    mod.set_axon_ntff_profile_hook = lambda h: _hook.__setitem__(0, h)
    mod.get_axon_ntff_profile_hook = lambda: _hook[0]
    antenv.axon_hooks = mod
    sys.modules["antenv.axon_hooks"] = mod

    lib = ctypes.CDLL("/opt/axon/libaxon_pjrt.so")
    if not hasattr(lib, "axon_start_nrt_profile"):
        return
    lib.axon_start_nrt_profile.argtypes = [ctypes.POINTER(ctypes.c_int64),
                                           ctypes.c_size_t]
    lib.axon_start_nrt_profile.restype = ctypes.c_int64
    lib.axon_stop_nrt_profile.argtypes = [ctypes.c_char_p]
    lib.axon_stop_nrt_profile.restype = ctypes.c_int64

    @contextlib.contextmanager
    def _profile(output_dir, device_ids):
        import jax

        jax.devices()
        if device_ids:
            ids = (ctypes.c_int64 * len(device_ids))(*device_ids)
            rc = lib.axon_start_nrt_profile(ids, len(device_ids))
        else:
            rc = lib.axon_start_nrt_profile(None, 0)
        if rc != 0:
            raise RuntimeError(f"axon_start_nrt_profile rc={rc}")
        try:
            yield
        finally:
            n = lib.axon_stop_nrt_profile(str(output_dir).encode())
            print(f"profile: {n} file(s) written to {output_dir}",
                  file=sys.stderr)

    mod.set_axon_ntff_profile_hook(_profile)
